# revision 29
# baseline (speedup 1.0000x reference)
"""Longformer attention Bass kernel for 8 TRN2 NeuronCores (v3, bf16).

Sharding: core c handles batch b = c//4 and heads 4*(c%4) .. 4*(c%4)+3.
Each core computes its 4 heads' attention + the partial output projection;
the host sums the 4 partials per batch element and adds the folded bias.

Design notes:
- all matmul operands bf16 (PSUM accumulates fp32): full-rate PE + FWL
- scores computed transposed (s^T [keys, queries]) so softmax needs no
  transposes: denominator comes from a ones-column appended to V and the
  per-query reciprocal is partition-broadcast on GpSimd
- phase 2 runs two heads interleaved with a one-item PV lag, and idle PE
  slots are filled with deferred projection / output chains sharing the
  same PSUM pool, so the PE never waits on the exp chain
- reciprocal_approx_fast needs its input at partition 0 (the HW ucode
  ignores AP partition offsets) -> the denominator row is staged to
  partition 0 with a plain DVE copy first
"""

import os
import numpy as np
import ml_dtypes

import concourse.bass as bass
import concourse.mybir as mybir
import concourse.tile as tile
from concourse import bacc
from concourse.bass_utils import run_bass_kernel_spmd

# ---- problem constants (hardcoded per contract) ----
B, S, DM = 2, 2048, 1024
H, DH = 16, 64
WINDOW = 128
NG = max(1, int(S * 0.1))  # 204 global tokens
SCALE = 1.0 / np.sqrt(DH)
NCORES = 8
HPC = 4            # heads per core
F = HPC * DH       # 256 per-core head features
KB = S // 128      # 16 key blocks

FP = mybir.dt.float32
BF = mybir.dt.bfloat16
AF = mybir.ActivationFunctionType
BF_NP = ml_dtypes.bfloat16


# ---------------------------------------------------------------- planning
def _allow():
    pos = np.arange(S)
    dist = pos[None, :] - pos[:, None]
    window = np.abs(dist) <= WINDOW // 2
    isg = pos < NG
    return window | isg[:, None] | isg[None, :]  # [query i, key j]


def _keyset(g):
    if g == 0:
        return list(range(KB))
    s = {0, 1}
    for c in range(2 * g - 1, 2 * g + 3):
        if 0 <= c < KB:
            s.add(c)
    return sorted(s)


def _plan3():
    """Per-half list of piece items.

    item: dict(c, w, q0, ops, pvs)
      ops: ('mul', mask_idx, o) or ('memset', p0, p1, c0, c1)  (piece-rel)
      pvs: [(o, ln, pair, off, start, stop)]  o piece-rel, pair 0..3 global
    """
    allowT = _allow().T  # [key, query]
    last_c = [max(_keyset(g)) for g in range(8)]
    masks, midx = [], {}

    def mask_id(sub):
        key = sub.tobytes()
        if key not in midx:
            midx[key] = len(masks)
            masks.append(sub.astype(np.float32))
        return midx[key]

    halves = []
    for half in (0, 1):
        items = []
        for c in range(KB):
            keysl = slice(c * 128, (c + 1) * 128)
            G = [g for g in range(4 * half, 4 * half + 4)
                 if allowT[keysl, 256 * g:256 * g + 256].any()]
            if not G:
                continue
            runs = []
            for g in G:
                if runs and runs[-1][0] + runs[-1][1] == 256 * g:
                    runs[-1][1] += 256
                else:
                    runs.append([256 * g, 256])
            pieces = []
            for q0, w in runs:
                o = 0
                while w > 0:
                    pw = min(512, w)
                    pieces.append((q0 + o, pw))
                    o += pw
                    w -= pw
            for q0, w in pieces:
                # trim trailing all-masked query columns off standalone
                # global pieces (not c==15: it carries pair-0's stop)
                if q0 == 0 and w == 256 and c >= 5 and c != 15:
                    sub = allowT[keysl, 0:256]
                    last = int(np.nonzero(sub.any(axis=0))[0].max()) + 1
                    w = min(w, (last + 3) & ~3)
                # block-1 pieces beyond any window overlap only see the
                # global keys 128..203: restrict to 76 key partitions so
                # no mask is needed
                k0, k1 = 0, 128
                if c == 1 and q0 >= 384:
                    k1 = NG - 128
                keysl = slice(c * 128 + k0, c * 128 + k1)
                ops = []
                for o in range(0, w, 256):
                    ow = min(256, w - o)
                    sub = allowT[keysl, q0 + o:q0 + o + ow]
                    if sub.all():
                        continue
                    assert ow == 256, (c, q0, w)
                    rfull = sub.all(axis=1)
                    rnone = ~sub.any(axis=1)
                    cfull = sub.all(axis=0)
                    cnone = ~sub.any(axis=0)
                    done = False
                    if (rfull | rnone).all() and rnone.any():
                        (idx,) = np.nonzero(rnone)
                        p0, p1 = int(idx.min()), int(idx.max() + 1)
                        if (rnone[p0:p1].all() and p1 - p0 == len(idx)
                                and p0 % 32 == 0 and p1 % 32 == 0):
                            ops.append(("memset", p0, p1, o, o + 256))
                            done = True
                    if not done and (cfull | cnone).all() and cnone.any():
                        (idx,) = np.nonzero(cnone)
                        c0, c1 = int(idx.min()), int(idx.max() + 1)
                        if cnone[c0:c1].all() and c1 - c0 == len(idx):
                            ops.append(("memset", 0, 128, o + c0, o + c1))
                            done = True
                    if not done:
                        ops.append(("mul", mask_id(sub), o))
                segs = []
                for o in range(0, w, 256):
                    ln = min(256, w - o)
                    g = (q0 + o) // 256
                    segs.append([o, ln, g // 2, (q0 + o) % 512,
                                 c == 0, c == last_c[g]])
                merged = [segs[0]]
                for sg in segs[1:]:
                    m = merged[-1]
                    if (sg[2] == m[2] and sg[4] == m[4] and sg[5] == m[5]
                            and m[0] + m[1] == sg[0] and m[3] + m[1] == sg[3]):
                        m[1] += sg[1]
                    else:
                        merged.append(sg)
                items.append(dict(c=c, w=w, q0=q0, k0=k0, k1=k1, ops=ops,
                                  pvs=[tuple(x) for x in merged]))
        halves.append(items)
    return halves, np.stack(masks)


# ---------------------------------------------------------------- builder
def _emit(tc, halves, nm, aps):
    nc = tc.nc
    hT, wqT, wkT, wvT, woT, bq2, bk2, mks, out = aps

    with tc.tile_pool(name="const", bufs=1) as const:
        hT_sb = const.tile([128, 8, S], BF, tag="hT")
        wq_sb = const.tile([128, 8, F], BF, tag="wq")
        wk_sb = const.tile([128, 8, F], BF, tag="wk")
        wv_sb = const.tile([128, 8, F], BF, tag="wv")
        wo_sb = const.tile([128, 2, DM], BF, tag="wo")
        bq_sb = const.tile([128, 2], FP, tag="bq")
        bk_sb = const.tile([128, 2], FP, tag="bk")
        mk_sb = const.tile([128, nm, 256], BF, tag="mk")
        qT_sb = const.tile([128, 2, S], BF, tag="qT")
        kT_sb = const.tile([128, 2, S], BF, tag="kT")
        v_sb = const.tile([128, HPC, KB, DH + 1], BF, tag="v")
        cT_sb = const.tile([128, 2, S], BF, tag="cT")

        # per-kt weight/hT chunks spread across DMA queues so the first
        # projection chain's inputs land as early as possible
        hT_r = hT.rearrange("(t p) n -> p t n", p=128)
        wq_r = wqT.rearrange("(t p) f -> p t f", p=128)
        wk_r = wkT.rearrange("(t p) f -> p t f", p=128)
        wv_r = wvT.rearrange("(t p) f -> p t f", p=128)
        for kt in range(8):
            nc.sync.dma_start(wq_sb[:, kt, :], wq_r[:, kt, :])
            nc.sync.dma_start(hT_sb[:, kt, 0:512], hT_r[:, kt, 0:512])
        for kt in range(8):
            nc.sync.dma_start(wk_sb[:, kt, :], wk_r[:, kt, :])
            nc.sync.dma_start(wv_sb[:, kt, :], wv_r[:, kt, :])
        nc.sync.dma_start(bq_sb[:], bq2.rearrange("t p -> p t"))
        nc.sync.dma_start(bk_sb[:], bk2.rearrange("t p -> p t"))
        for ntq in range(1, 4):
            for kt in range(8):
                nc.sync.dma_start(
                    hT_sb[:, kt, ntq * 512:(ntq + 1) * 512],
                    hT_r[:, kt, ntq * 512:(ntq + 1) * 512])
        nc.sync.dma_start(wo_sb[:], woT.rearrange("(t p) f -> p t f", p=128))
        nc.sync.dma_start(mk_sb[:], mks.rearrange("n p m -> p n m"))
        nc.vector.memset(v_sb[:, :, :, DH:DH + 1], 1.0)

        with tc.tile_pool(name="ps", bufs=4, space="PSUM") as psp, \
             tc.tile_pool(name="pctx", bufs=4, space="PSUM") as pctx, \
             tc.tile_pool(name="work", bufs=8) as work, \
             tc.tile_pool(name="rcb", bufs=2) as rcbp, \
             tc.tile_pool(name="rcp", bufs=2) as rcp, \
             tc.tile_pool(name="ostg", bufs=4) as ostg:

            def chain(kind, *args):
                """Deferred projection / output chains, all on one pool."""
                ps = psp.tile([128, 512], FP, tag="sT",
                              name=f"ch_{kind}_{args}")
                if kind in ("q", "k"):
                    mt, nt = args
                    wsb, bsb, dst = ((wq_sb, bq_sb, qT_sb) if kind == "q"
                                     else (wk_sb, bk_sb, kT_sb))
                    for kt in range(8):
                        nc.tensor.matmul(
                            ps[:], wsb[:, kt, mt * 128:(mt + 1) * 128],
                            hT_sb[:, kt, nt * 512:(nt + 1) * 512],
                            start=(kt == 0), stop=(kt == 7))
                    nc.vector.tensor_scalar_add(
                        dst[:, mt, nt * 512:(nt + 1) * 512], ps[:],
                        bsb[:, mt:mt + 1])
                elif kind == "v":
                    st = args[0]  # covers st, st+1
                    for sub in range(2):
                        for kt in range(8):
                            nc.tensor.matmul(
                                ps[:, sub * 256:(sub + 1) * 256],
                                hT_sb[:, kt, (st + sub) * 128:
                                      (st + sub + 1) * 128],
                                wv_sb[:, kt, :],
                                start=(kt == 0), stop=(kt == 7))
                    nc.scalar.activation(
                        v_sb[:, :, st:st + 2, 0:DH],
                        ps.rearrange("p (c h d) -> p h c d", c=2, h=HPC),
                        AF.Copy)
                else:  # "po"
                    st, nt = args
                    for kt in range(2):
                        nc.tensor.matmul(
                            ps[:], cT_sb[:, kt, st * 128:(st + 1) * 128],
                            wo_sb[:, kt, nt * 512:(nt + 1) * 512],
                            start=(kt == 0), stop=(kt == 1))
                    ot = ostg.tile([128, 512], FP, tag="ot",
                                   name=f"ot{st}_{nt}")
                    if (st * 2 + nt) % 2 == 0:
                        nc.scalar.activation(ot[:], ps[:], AF.Copy)
                    else:
                        nc.vector.tensor_copy(ot[:], ps[:])
                    nc.sync.dma_start(
                        out[st * 128:(st + 1) * 128,
                            nt * 512:(nt + 1) * 512], ot[:])

            # minimal prefix so (half0, heads 0/1) can start
            for ck in (("q", 0, 0), ("q", 0, 1), ("k", 0, 0),
                       ("v", 0), ("v", 2)):
                chain(*ck)

            fillers = {
                (0, 0): [("k", 0, 1), ("v", 4), ("k", 0, 2), ("v", 6),
                         ("k", 0, 3), ("v", 8), ("v", 10), ("v", 12),
                         ("v", 14), ("q", 0, 2), ("q", 0, 3), ("q", 1, 0),
                         ("q", 1, 1), ("k", 1, 0), ("k", 1, 1), ("k", 1, 2),
                         ("k", 1, 3)],
                (0, 1): [("q", 1, 2), ("q", 1, 3)],
                (1, 0): [("po", st, nt) for st in range(4)
                         for nt in range(2)],
                (1, 1): [("po", st, nt) for st in range(4, 8)
                         for nt in range(2)],
            }

            for half in (0, 1):
                items = halves[half]
                pair_done = {}
                for i, it in enumerate(items):
                    for pv in it["pvs"]:
                        pair_done[pv[2]] = i
                for hp in (0, 1):
                    heads = (2 * hp, 2 * hp + 1)
                    fq = list(fillers[(half, hp)])
                    ctx = {}
                    for h in heads:
                        for lp in sorted({pv[2] for it in items
                                          for pv in it["pvs"]}):
                            ctx[(h, lp)] = pctx.tile(
                                [DH + 1, 512], FP, tag="ctx",
                                name=f"ctx{half}_{h}_{lp}")
                    exs = {}

                    def pv_block(i):
                        it = items[i]
                        c, k0, k1 = it["c"], it["k0"], it["k1"]
                        for h in heads:
                            ex = exs.pop((h, i))
                            for (o, ln, lp, off, st_, sp_) in it["pvs"]:
                                nc.tensor.matmul(
                                    ctx[(h, lp)][:, off:off + ln],
                                    v_sb[k0:k1, h, c, :],
                                    ex[0:k1 - k0, o:o + ln],
                                    start=st_, stop=sp_,
                                    skip_group_check=True)
                        for lp, di in pair_done.items():
                            if di != i:
                                continue
                            for h in heads:
                                p0, mt = 64 * (h % 2), h // 2
                                den = rcp.tile([1, 512], FP, tag="den",
                                               name=f"den{half}_{h}_{lp}")
                                nc.vector.tensor_copy(
                                    den[:], ctx[(h, lp)][DH:DH + 1, :])
                                rc = rcp.tile([1, 512], FP, tag="rc",
                                              name=f"rc{half}_{h}_{lp}")
                                nc.vector.reciprocal_approx_fast(rc[:],
                                                                 den[:])
                                rcb = rcbp.tile([DH, 512], FP, tag="rcb",
                                                name=f"rcb{half}_{h}_{lp}")
                                nc.gpsimd.partition_broadcast(rcb[:], rc[:],
                                                              channels=DH)
                                dst = cT_sb[p0:p0 + 64, mt,
                                            lp * 512:(lp + 1) * 512]
                                nc.vector.tensor_mul(
                                    dst, ctx[(h, lp)][0:DH, :], rcb[:])

                    pending = None
                    for i, it in enumerate(items):
                        c, w, q0 = it["c"], it["w"], it["q0"]
                        k0, k1 = it["k0"], it["k1"]
                        km = k1 - k0
                        for h in heads:
                            p0, mt = 64 * (h % 2), h // 2
                            ps = psp.tile([128, 512], FP, tag="sT",
                                          name=f"sT{half}_{h}_{c}_{q0}")
                            nc.tensor.matmul(
                                ps[0:km, 0:w],
                                kT_sb[p0:p0 + 64, mt,
                                      c * 128 + k0:c * 128 + k1],
                                qT_sb[p0:p0 + 64, mt, q0:q0 + w],
                                start=True, stop=True)
                            ex = work.tile([128, 512], BF, tag="ex",
                                           name=f"ex{half}_{h}_{c}_{q0}")
                            nc.scalar.activation(ex[0:km, 0:w],
                                                 ps[0:km, 0:w], AF.Exp)
                            for op in it["ops"]:
                                if op[0] == "mul":
                                    _, mi, o = op
                                    nc.vector.tensor_mul(
                                        ex[:, o:o + 256], ex[:, o:o + 256],
                                        mk_sb[:, mi, :])
                                else:
                                    _, r0, r1, c0, c1 = op
                                    nc.vector.memset(ex[r0:r1, c0:c1], 0.0)
                            exs[(h, i)] = ex
                        if pending is not None:
                            pv_block(pending)
                        if fq:
                            chain(*fq.pop(0))
                        pending = i
                    pv_block(pending)
                    while fq:
                        chain(*fq.pop(0))

            # ---- phase 3 tail: remaining output projection
            for st in range(8, KB):
                for nt in range(2):
                    chain("po", st, nt)


_CACHE = {}
TRACE_KWARGS = {}  # test harness may set e.g. dict(tmpdir=...)


def _get_nc():
    if "nc" in _CACHE:
        return _CACHE["nc"], _CACHE["masks"]
    halves, masks = _plan3()
    nm = masks.shape[0]
    nc = bacc.Bacc("TRN2", target_bir_lowering=False, debug=False,
                   enable_asserts=False)

    def dp(name, shape, dtype=BF, is_out=False):
        h = nc.declare_dram_parameter(name, list(shape), dtype, isOutput=is_out)
        return h[:]

    aps = (
        dp("hT", [DM, S]),
        dp("wqT", [DM, F]),
        dp("wkT", [DM, F]),
        dp("wvT", [DM, F]),
        dp("woT", [F, DM]),
        dp("bq2", [2, 128], FP),
        dp("bk2", [2, 128], FP),
        dp("mks", [nm, 128, 256]),
        dp("out", [S, DM], FP, True),
    )
    with tile.TileContext(nc) as tc:
        _emit(tc, halves, nm, aps)
    nc.compile()
    _CACHE["nc"] = nc
    _CACHE["masks"] = masks
    return nc, masks


def make_in_maps(hidden_states, Wq, bq, Wk, bk, Wv, bv, Wo, bo, masks):
    in_maps = []
    f32 = np.float32
    mks_bf = masks.astype(BF_NP)
    for core in range(NCORES):
        b, fs = core // 4, (core % 4) * F
        in_maps.append({
            "hT": np.ascontiguousarray(hidden_states[b].T).astype(BF_NP),
            "wqT": np.ascontiguousarray((Wq[fs:fs + F] * SCALE).T).astype(BF_NP),
            "wkT": np.ascontiguousarray(Wk[fs:fs + F].T).astype(BF_NP),
            "wvT": np.ascontiguousarray(Wv[fs:fs + F].T).astype(BF_NP),
            "woT": np.ascontiguousarray(Wo[:, fs:fs + F].T).astype(BF_NP),
            "bq2": (bq[fs:fs + F] * SCALE).reshape(2, 128).astype(f32),
            "bk2": bk[fs:fs + F].reshape(2, 128).astype(f32),
            "mks": mks_bf,
        })
    return in_maps


def kernel(hidden_states, Wq, bq, Wk, bk, Wv, bv, Wo, bo):
    nc, masks = _get_nc()
    in_maps = make_in_maps(hidden_states, Wq, bq, Wk, bk, Wv, bv, Wo, bo,
                           masks)
    trace = bool(int(os.environ.get("ATTN_TRACE", "0")))
    kw = dict(TRACE_KWARGS) if trace else {}
    res = run_bass_kernel_spmd(nc, in_maps, core_ids=list(range(NCORES)),
                               trace=trace, **kw)
    _CACHE["last_results"] = res
    bias = (bo + Wo @ bv).astype(np.float32)
    out = np.empty((B, S, DM), np.float32)
    for b in range(B):
        acc = res.results[4 * b]["out"].astype(np.float32).copy()
        for c in range(4 * b + 1, 4 * b + 4):
            acc += res.results[c]["out"]
        out[b] = acc + bias
    return out
